# revision 26
# baseline (speedup 1.0000x reference)
# Trainium2 Bass kernel for nn_PredLayer (vocab-parallel prediction layer +
# label-smoothed KL loss).
#
#   scores = x @ W.T + b              [N_TOK, VOCAB]
#   loss   = KLDiv(log_softmax(scores), smoothed_one_hot(labels)).sum() / N_TOK
#
# Strategy (8 NeuronCores, tensor-parallel over the vocab dim):
#   - host: pad vocab to 8 * 6400, transpose W shard -> [D, V_PAD] per core,
#     transpose x -> [D, N_TOK] (replicated).
#   - device c: scores_c = x @ W_c.T + b_c   (f32r matmuls, PSUM accumulate
#     over D), fused bias-add + row-sum (DVE tensor_tensor_reduce) and fused
#     exp + row-sum-of-exp (ACT activation accum_out). Emits the scores shard
#     plus per-token partial rowsum / sum-exp stats.
#   - host: concatenate score shards, combine the tiny per-core stats into
#     log-sum-exp, gather label/pad scores, assemble the scalar loss.
#
# Self-contained: hardcodes all shapes from the problem spec.

import functools

import numpy as np

import concourse.bass as bass
import concourse.mybir as mybir
import concourse.tile as tile
from concourse import bacc, bass_utils

F32 = mybir.dt.float32
F32R = mybir.dt.float32r

# Problem constants (hardcoded from spec).
N_TOK = 2048
D_MODEL = 1024
VOCAB = 50257
LABEL_SMOOTHING = 0.1
PAD_ID = 1
N_CORES = 8

P = 128  # SBUF partitions

# Vocab sharding: ceil(50257/8) = 6283 real columns per core (last core 6276).
# Padded to 6284, chunked 11*512 + 326 + 326: every chunk fits one PSUM bank,
# has free dim >= 256 (full-rate f32r) and an even width (f32r ISA rule).
V_SHARD = 6283
V_PAD = 6284
CHUNK = 512
# Padded weight rows are zero and padded bias is PAD_BIAS, so padded score
# columns are exactly PAD_BIAS: exp(PAD_BIAS) == 0 (keeps sum-exp clean) and
# the known PAD_BIAS * n_pad contribution to rowsum is corrected on the host.
PAD_BIAS = -100.0


def _chunks(v_pad, chunk):
    """Split v_pad into chunks of `chunk`, keeping every chunk >= 256
    (f32r full-rate) and even-width (f32r ISA rule) by splitting any short
    tail across two chunks."""
    assert v_pad % 2 == 0
    out = []
    v0 = 0
    while v0 < v_pad:
        rem = v_pad - v0
        if chunk < rem < chunk + 256:
            a = (rem // 2 + 1) // 2 * 2  # even, >= rem/2
            out.append((v0, a))
            out.append((v0 + a, rem - a))
            v0 = v_pad
        else:
            vlen = min(chunk, rem)
            out.append((v0, vlen))
            v0 += vlen
    assert sum(c[1] for c in out) == v_pad
    assert all(c[1] >= 256 and c[1] % 2 == 0 for c in out) or v_pad < 256
    return out


@functools.lru_cache(maxsize=2)
def build_nc(
    n_tok=N_TOK,
    d=D_MODEL,
    v_pad=V_PAD,
    chunk=CHUNK,
    wp_bufs=2,
    sp_bufs=2,
    ps_bufs=8,
    ep_bufs=4,
    m_group=4,
    no_exp=False,
    no_out=False,
    no_add=False,
):
    """Build + compile the per-core Bass program (identical on all cores)."""
    m_tiles = n_tok // P
    k_tiles = d // P
    chunks = _chunks(v_pad, chunk)
    n_chunks = len(chunks)
    if m_group is None:
        m_group = 8 if m_tiles % 8 == 0 else m_tiles  # m-tiles per output DMA
    m_group = min(m_group, m_tiles)

    nc = bacc.Bacc(
        "TRN2", target_bir_lowering=False, debug=False, num_devices=N_CORES
    )
    xT = nc.dram_tensor("xT", [d, n_tok], F32R, kind="ExternalInput").ap()
    wT = nc.dram_tensor("wT", [d, v_pad], F32R, kind="ExternalInput").ap()
    bv = nc.dram_tensor("bv", [v_pad], F32, kind="ExternalInput").ap()
    scores = nc.dram_tensor("scores", [n_tok, v_pad], F32, kind="ExternalOutput").ap()
    # stats[p, m] = sumexp of token m*128+p
    stats = nc.dram_tensor("stats", [P, m_tiles], F32, kind="ExternalOutput").ap()

    with tile.TileContext(nc) as tc:
        with (
            tc.tile_pool(name="xp", bufs=1) as xp,
            tc.tile_pool(name="wp", bufs=wp_bufs) as wp,
            tc.tile_pool(name="sp", bufs=sp_bufs) as sp,
            tc.tile_pool(name="bp", bufs=1) as bp,
            tc.tile_pool(name="ep", bufs=ep_bufs) as ep,
            tc.tile_pool(name="acc", bufs=1) as acc,
            tc.tile_pool(name="ps", bufs=ps_bufs, space="PSUM") as psp,
        ):
            # x.T resident in SBUF: [p, k, n] with d = k*128 + p.
            # Split per k-tile so the first matmuls can start as soon as the
            # k=0 slice lands instead of waiting for the full 8 MB. Chunk 0's
            # weight panel is split per-k too and interleaved with the xT
            # slices so matmul k can start after ~ (k+1) * 1.25 MB of DMA.
            xt = xp.tile([P, k_tiles, n_tok], F32R)
            xT_r = xT.rearrange("(k p) n -> p k n", p=P)
            v0_0, vlen_0 = chunks[0]
            wt0 = wp.tile([P, k_tiles, vlen_0], F32R, tag="wt")
            wT0_r = wT[:, v0_0 : v0_0 + vlen_0].rearrange("(k p) v -> p k v", p=P)
            for k in range(k_tiles):
                nc.sync.dma_start(out=wt0[:, k, :], in_=wT0_r[:, k, :])
                nc.sync.dma_start(out=xt[:, k, :], in_=xT_r[:, k, :])

            # bias: one 25 KB DMA + on-chip partition broadcast (keeps the
            # startup DMA queue clear of a 3.2 MB replicated transfer)
            b_row = bp.tile([1, v_pad], F32, tag="b1")
            nc.gpsimd.dma_start(out=b_row[:], in_=bv[None, :])
            bsb = bp.tile([P, v_pad], F32, tag="bP")
            nc.gpsimd.partition_broadcast(bsb[:], b_row[:])

            se_acc = acc.tile([P, m_tiles * n_chunks], F32, tag="se")
            stats_sb = acc.tile([P, m_tiles], F32, tag="st")

            def epilogue(ps, m, j, v0, vlen, sb, mi):
                col = m * n_chunks + j
                # scores_sb = psum + b
                if no_add:
                    nc.vector.tensor_copy(sb[:, mi, :], ps[:])
                else:
                    nc.vector.tensor_add(sb[:, mi, :], ps[:], bsb[:, v0 : v0 + vlen])
                if not no_exp:
                    # exp(scores) -> scratch ; sumexp partial
                    ex = ep.tile([P, vlen], F32, tag="ex")
                    nc.scalar.activation(
                        out=ex[:],
                        in_=sb[:, mi, :],
                        func=mybir.ActivationFunctionType.Exp,
                        accum_out=se_acc[:, col : col + 1],
                    )
                    if j == n_chunks - 1:
                        # last chunk: fold this token-tile's per-chunk sumexp
                        # partials right away (overlaps remaining matmuls)
                        nc.vector.tensor_reduce(
                            out=stats_sb[:, m : m + 1],
                            in_=se_acc[:, m * n_chunks : (m + 1) * n_chunks],
                            axis=mybir.AxisListType.X,
                            op=mybir.AluOpType.add,
                        )

            def out_dma(sb, mg, group, j, v0, vlen):
                if not no_out:
                    nc.sync.dma_start(
                        out=scores[
                            mg * group * P : (mg + 1) * group * P, v0 : v0 + vlen
                        ].rearrange("(mi p) v -> p mi v", p=P),
                        in_=sb[:],
                    )

            for j, (v0, vlen) in enumerate(chunks):
                if j == 0:
                    wt = wt0
                else:
                    wt = wp.tile([P, k_tiles, vlen], F32R, tag="wt")
                    nc.sync.dma_start(
                        out=wt[:],
                        in_=wT[:, v0 : v0 + vlen].rearrange("(k p) v -> p k v", p=P),
                    )
                if j == 0:
                    # k-outer in groups of 4 m-tiles: matmuls for k-tile k run
                    # as soon as slice k of xT/wT0 lands (PE overlaps the
                    # initial x.T load instead of waiting for all 8 MB).
                    group = min(4, m_tiles)
                    for mg in range(m_tiles // group):
                        sb = sp.tile([P, group, vlen], F32, tag="sb0")
                        pss = [
                            psp.tile([P, vlen], F32, tag="ps", name=f"ps0_{mg}_{i}")
                            for i in range(group)
                        ]
                        for k in range(k_tiles):
                            for mi in range(group):
                                m = mg * group + mi
                                nc.tensor.matmul(
                                    pss[mi][:],
                                    lhsT=xt[:, k, m * P : (m + 1) * P],
                                    rhs=wt[:, k, :],
                                    start=(k == 0),
                                    stop=(k == k_tiles - 1),
                                )
                        for mi in range(group):
                            epilogue(pss[mi], mg * group + mi, j, v0, vlen, sb, mi)
                        out_dma(sb, mg, group, j, v0, vlen)
                else:
                    for mg in range(m_tiles // m_group):
                        sb = sp.tile([P, m_group, vlen], F32, tag="sb")
                        for mi in range(m_group):
                            m = mg * m_group + mi
                            ps = psp.tile([P, vlen], F32, tag="ps")
                            for k in range(k_tiles):
                                nc.tensor.matmul(
                                    ps[:],
                                    lhsT=xt[:, k, m * P : (m + 1) * P],
                                    rhs=wt[:, k, :],
                                    start=(k == 0),
                                    stop=(k == k_tiles - 1),
                                )
                            epilogue(ps, m, j, v0, vlen, sb, mi)
                        out_dma(sb, mg, m_group, j, v0, vlen)

            if no_exp:
                nc.vector.memset(stats_sb[:], 0.0)
            nc.sync.dma_start(out=stats[:, :], in_=stats_sb[:])

    nc.compile()
    return nc


def _shard_inputs(x, W, b):
    """Host-side sharding: per-core transposed weight shard + padded bias."""
    x = np.ascontiguousarray(np.asarray(x, dtype=np.float32))
    W = np.asarray(W, dtype=np.float32)
    b = np.asarray(b, dtype=np.float32)

    xT = np.ascontiguousarray(x.T)  # [D, N_TOK]

    in_maps = []
    meta = []
    for c in range(N_CORES):
        s = c * V_SHARD
        e = min(VOCAB, s + V_SHARD)
        real = e - s
        wTc = np.zeros((D_MODEL, V_PAD), dtype=np.float32)
        # blocked transpose of the shard
        wTc[:, :real] = W[s:e, :].T
        bc = np.full((V_PAD,), PAD_BIAS, dtype=np.float32)
        bc[:real] = b[s:e]
        in_maps.append({"xT": xT, "wT": wTc, "bv": bc})
        meta.append((s, e, real))
    return in_maps, meta


def _combine(results, meta, labels, rowsum):
    """Host-side gather: assemble full scores + scalar loss."""
    n = N_TOK
    labels = np.asarray(labels)
    scores_full = np.empty((n, VOCAB), dtype=np.float32)
    sumexp = np.zeros(n, dtype=np.float64)
    m_tiles = n // P
    for c, (s, e, real) in enumerate(meta):
        r = results[c]
        scores_full[:, s:e] = r["scores"][:, :real]
        st = r["stats"]  # [P, m_tiles]; token index = m*128 + p
        sumexp += st.T.reshape(n).astype(np.float64)

    v = float(VOCAB)
    confidence = 1.0 - LABEL_SMOOTHING
    sv = LABEL_SMOOTHING / (v - 2.0)

    lse = np.log(sumexp)  # no max-sub needed: scores are O(5)
    idx = np.arange(n)
    s_lab = scores_full[idx, labels].astype(np.float64)
    s_pad = scores_full[:, PAD_ID].astype(np.float64)

    h_const = confidence * np.log(confidence) + (v - 2.0) * sv * np.log(sv)
    kl = h_const + lse - confidence * s_lab - sv * (rowsum - s_pad - s_lab)
    kl = np.where(labels == PAD_ID, 0.0, kl)
    loss = np.float32(kl.sum() / n)
    return scores_full, loss


def _run(in_maps, trace=False):
    nc = build_nc()
    last_exc = None
    for _ in range(2):
        try:
            return bass_utils.run_bass_kernel_spmd(
                nc,
                in_maps,
                core_ids=list(range(N_CORES)),
                trace=trace,
            )
        except Exception as e:  # rare transient device/terminal failures
            last_exc = e
    raise last_exc


def _host_rowsum(x, W, b):
    """rowsum_n = sum_v scores[n, v] = x_n . W.sum(0) + b.sum().

    Exact (f64) evaluation of the smoothing term's score-sum; it enters the
    loss scaled by smoothing/(V-2) ~ 2e-6, so any rounding-level difference
    vs the reference's f32 sum is far below tolerance.
    """
    w_sum = np.asarray(W, dtype=np.float32).sum(axis=0, dtype=np.float64)
    b_sum = np.asarray(b, dtype=np.float64).sum()
    return np.asarray(x, dtype=np.float32).astype(np.float64) @ w_sum + b_sum


def kernel(x, W, b, labels):
    in_maps, meta = _shard_inputs(x, W, b)
    rowsum = _host_rowsum(x, W, b)
    res = _run(in_maps)
    return _combine(res.results, meta, labels, rowsum)


def kernel_with_results(x, W, b, labels, trace=False):
    """Like kernel() but also returns the BassKernelResults (for profiling)."""
    in_maps, meta = _shard_inputs(x, W, b)
    rowsum = _host_rowsum(x, W, b)
    res = _run(in_maps, trace=trace)
    return _combine(res.results, meta, labels, rowsum), res
